# revision 4
# baseline (speedup 1.0000x reference)
"""Diagonal-scale kernel: y = x * |diag(W)|, distributed over 8 NeuronCores.

x: [65536, 1024] f32, W: [1024, 1024] f32 -> y: [65536, 1024] f32.
Pure data parallel: each core handles a contiguous [8192, 1024] slice of x;
the 1024-element |diag(W)| vector is replicated across all 128 SBUF
partitions on the host and broadcast to every core.

Raw Bass pipeline (Tile's auto-sems emit multi-wait compute instructions
that this walrus build rejects):
  SP  engine: HWDGE loads   x tile t -> SBUF slot t % NSLOTS
  DVE engine: in-place tensor_mul by the replicated diag vector
  ACT engine: HWDGE stores  slot -> y tile t
Slot reuse is gated with per-slot semaphores so every instruction needs
at most one sync wait.
"""

from contextlib import ExitStack

import numpy as np

NCORES = 8
B, N = 65536, 1024
BL = B // NCORES  # 8192 rows per core
P = 128           # SBUF partitions
R = 8             # consecutive x rows per partition per tile
F = R * N         # free elems per partition per tile (32 KB)
TILES = BL // (P * R)  # 8 tiles of [128, 8192] f32 (4 MB) per core
NSLOTS = 3

_cached_nc = None
TRACE = False
TRACE_KWARGS = {}
LAST_RESULT = None


def _build(r=None, nslots=None):
    global R, F, TILES, NSLOTS
    if r is not None:
        R, F, TILES = r, r * N, BL // (P * r)
    if nslots is not None:
        NSLOTS = nslots
    return _build_inner()


def _build_inner():
    import concourse.bass as bass
    import concourse.mybir as mybir

    f32 = mybir.dt.float32
    nc = bass.Bass("TRN2", debug=False, num_devices=NCORES)
    x = nc.dram_tensor("x", [BL, N], f32, kind="ExternalInput")
    wd = nc.dram_tensor("wd", [P, N], f32, kind="ExternalInput")
    y = nc.dram_tensor("y", [BL, N], f32, kind="ExternalOutput")

    # Tile t, partition p holds R consecutive rows -> 32 KB contiguous DRAM
    # per partition, one 4 MB dma_start per tile.
    xv = x.ap().rearrange("(t p r) m -> t p (r m)", p=P, r=R)
    yv = y.ap().rearrange("(t p r) m -> t p (r m)", p=P, r=R)

    with ExitStack() as ctx:
        block = ctx.enter_context(nc.Block())
        wt = ctx.enter_context(nc.sbuf_tensor("wt", [P, N], f32))
        xt = ctx.enter_context(nc.sbuf_tensor("xt", [P, NSLOTS * F], f32))
        wt_sem = ctx.enter_context(nc.semaphore("wt_sem"))
        dve_sem = ctx.enter_context(nc.semaphore("dve_sem"))
        in_sems = [
            ctx.enter_context(nc.semaphore(f"in_sem{s}")) for s in range(NSLOTS)
        ]
        out_sems = [
            ctx.enter_context(nc.semaphore(f"out_sem{s}")) for s in range(NSLOTS)
        ]

        @block.sync
        def _(sync):
            sync.dma_start(wt[:], wd.ap()).then_inc(wt_sem, 16)
            for t in range(TILES):
                s = t % NSLOTS
                if t >= NSLOTS:
                    # Slot reuse: wait until the store of tile t-NSLOTS drained.
                    sync.wait_ge(out_sems[s], 16 * (t // NSLOTS))
                sync.dma_start(
                    xt[:, s * F : (s + 1) * F], xv[t]
                ).then_inc(in_sems[s], 16)

        @block.vector
        def _(vector):
            vector.wait_ge(wt_sem, 16)
            for t in range(TILES):
                s = t % NSLOTS
                vector.wait_ge(in_sems[s], 16 * (t // NSLOTS + 1))
                for j in range(R):
                    col = s * F + j * N
                    vector.tensor_mul(
                        xt[:, col : col + N], xt[:, col : col + N], wt[:]
                    ).then_inc(dve_sem, 1)

        @block.scalar
        def _(act):
            for t in range(TILES):
                s = t % NSLOTS
                act.wait_ge(dve_sem, R * (t + 1))
                act.dma_start(yv[t], xt[:, s * F : (s + 1) * F]).then_inc(
                    out_sems[s], 16
                )
            # Don't let the program end while stores are still in flight.
            for s in range(NSLOTS):
                n_stores = len([t for t in range(TILES) if t % NSLOTS == s])
                act.wait_ge(out_sems[s], 16 * n_stores)

    return nc


def kernel(x, W):
    global _cached_nc, LAST_RESULT
    from concourse.bass_utils import run_bass_kernel_spmd

    if _cached_nc is None:
        _cached_nc = _build()
    nc = _cached_nc

    x = np.asarray(x, dtype=np.float32)
    W = np.asarray(W, dtype=np.float32)
    wdiag = np.abs(np.diagonal(W)).astype(np.float32)
    wd = np.ascontiguousarray(np.broadcast_to(wdiag, (P, N)))
    xs = np.ascontiguousarray(x).reshape(NCORES, BL, N)

    in_maps = [{"x": xs[i], "wd": wd} for i in range(NCORES)]
    res = run_bass_kernel_spmd(
        nc, in_maps, list(range(NCORES)), trace=TRACE, **TRACE_KWARGS
    )
    LAST_RESULT = res
    return np.concatenate([res.results[i]["y"] for i in range(NCORES)], axis=0)
